# revision 10
# baseline (speedup 1.0000x reference)
"""CGCN message-passing kernel for 8 Trainium2 NeuronCores (Bass/Tile).

Strategy
--------
Host (numpy) relabels nodes into degree-sorted blocks of 128 dst slots and
pads each slot's edge list to the block max degree, so on-device the
segmented softmax/scatter becomes dense [128, T, 64] tile ops with
free-dim reductions (no per-edge scatter at all).

- Edges are sharded by destination: each core owns 1/8 of the users
  (routing + final pass) and 1/8 of the items (final pass), so scatter
  outputs are core-local; the only collectives are two AllGathers
  (item features after the MLP, final user prefs before the last pass).
- x[col] rows are fetched with the Q7 `dma_gather` extended instruction
  (int16 indices, 256B rows) across 4 SWDGE queues.
- The 2048->64 MLP runs row-sharded over items with the bias folded into
  an extra ones-column (host side), fp32 matmuls on PE.
- Padding edges point at an all-zero table row: they contribute exp(0)=1
  to the softmax denominator, which is subtracted back out with a
  host-computed per-slot pad count.
"""
import sys

sys.path.insert(0, "/opt/trn_rl_repo")

import numpy as np

import concourse.bass as bass
import concourse.bacc as bacc
import concourse.mybir as mybir
import concourse.tile as tile

NCORE = 8
DBG = {"p0", "ag1", "p0b", "p1", "ag2", "p2"}
EPS_SOFT = 1e-16
NEG_SLOPE = 0.01
P = 128


# ----------------------------------------------------------------------------
# host-side preprocessing helpers
# ----------------------------------------------------------------------------

def _cumcount(keys):
    """t[e] = rank of e among equal keys (input order preserved)."""
    order = np.argsort(keys, kind="stable")
    ks = keys[order]
    starts = np.r_[0, np.nonzero(ks[1:] != ks[:-1])[0] + 1]
    group_of = np.cumsum(np.r_[0, (ks[1:] != ks[:-1]).astype(np.int64)])
    t_sorted = np.arange(len(keys)) - starts[group_of]
    t = np.empty_like(t_sorted)
    t[order] = t_sorted
    return t


def _wrap_idx(idx):
    """[N] -> dma_gather idx tile [128, N//16] int16 (wrapped + 8x replicated)."""
    n = idx.shape[0]
    assert n % 16 == 0
    w = np.zeros((16, n // 16), dtype=np.int16)
    w[np.arange(n) % 16, np.arange(n) // 16] = idx.astype(np.int16)
    return np.tile(w, (8, 1))


class _Plan:
    pass


def _preprocess(features, mlp_w, mlp_b, preference, adj, adj_user):
    pl = _Plan()
    NU = preference.shape[0]
    NI = features.shape[0]
    DF = features.shape[1]
    DC = mlp_w.shape[1]
    assert DC == 64

    UPC = ((NU + NCORE - 1) // NCORE + P - 1) // P * P   # user slots per core
    IPC = ((NI + NCORE - 1) // NCORE + P - 1) // P * P   # item slots per core
    NB1 = UPC // P
    NBI = IPC // P
    pl.NU, pl.NI, pl.DF, pl.DC = NU, NI, DF, DC
    pl.UPC, pl.IPC, pl.NB1, pl.NBI = UPC, IPC, NB1, NBI
    pl.ULO = (NCORE // 2) * UPC                 # rows in users-lo table
    pl.UHI = NCORE * UPC - pl.ULO
    pl.ZIT = NCORE * IPC                        # zero row of item table
    pl.ZUL = pl.ULO
    pl.ZUH = pl.UHI

    u1 = np.asarray(adj_user[0])
    it1 = np.asarray(adj_user[1]) - NU
    E1 = u1.shape[0]

    # ---- user assignment: degree-sorted round-robin over cores -------------
    deg1 = np.bincount(u1, minlength=NU)
    order_u = np.argsort(-deg1, kind="stable")
    core_u = np.empty(NU, np.int64)
    slot_u = np.empty(NU, np.int64)
    r = np.arange(NU)
    core_u[order_u] = r % NCORE
    slot_u[order_u] = r // NCORE
    pl.core_u, pl.slot_u = core_u, slot_u

    # per-block pad target T1[b] = max degree among users of that block
    blk_u = slot_u // P
    T1 = np.zeros(NB1, np.int64)
    np.maximum.at(T1, blk_u, deg1)
    T1 = np.maximum(T1, 1)
    pl.T1 = T1
    off1 = np.r_[0, np.cumsum(P * T1)]          # slot-array offsets per block
    cum1 = np.r_[0, np.cumsum(T1)]              # alpha-column offsets
    pl.off1, pl.cum1 = off1, cum1
    E1S = int(off1[-1])
    pl.E1S = E1S

    # ---- item assignment (lexsorted by (cntA, cntB) for tight padding) -----
    # final pass, item-dst half: cols are users; table A = cores 0..3
    deg2 = np.bincount(it1, minlength=NI)       # item-dst degree in adj
    colA = core_u[u1] < (NCORE // 2)
    cntA = np.bincount(it1, weights=colA.astype(np.float64), minlength=NI).astype(np.int64)
    cntB = deg2 - cntA
    order_i = np.lexsort((-cntB, -cntA))
    core_i = np.empty(NI, np.int64)
    slot_i = np.empty(NI, np.int64)
    r = np.arange(NI)
    core_i[order_i] = r % NCORE
    slot_i[order_i] = r // NCORE
    pl.core_i, pl.slot_i = core_i, slot_i

    blk_i = slot_i // P
    T2A = np.zeros(NBI, np.int64)
    T2B = np.zeros(NBI, np.int64)
    np.maximum.at(T2A, blk_i, cntA)
    np.maximum.at(T2B, blk_i, cntB)
    T2A = np.maximum(T2A, 1)
    T2B = np.maximum(T2B, 1)
    pl.T2A, pl.T2B = T2A, T2B
    off2A = np.r_[0, np.cumsum(P * T2A)]
    off2B = np.r_[0, np.cumsum(P * T2B)]
    cum2 = int(cum1[-1]) + np.r_[0, np.cumsum(T2A + T2B)]
    pl.off2A, pl.off2B, pl.cum2 = off2A, off2B, cum2
    pl.AW = int(cum2[-1])
    E2AS, E2BS = int(off2A[-1]), int(off2B[-1])
    pl.E2AS, pl.E2BS = E2AS, E2BS

    # item-table row of each item
    itrow = core_i * IPC + slot_i
    # user-table position of each user
    ug = core_u * UPC + slot_u

    # ---- P1 / P2-user-half edge placement (dst = user, col = item) ---------
    ec = core_u[u1]
    elane = slot_u[u1] % P
    eblk = blk_u[u1]
    et = _cumcount(u1)
    pos1 = off1[eblk] + et * P + elane           # position in per-core slot array
    idx1 = np.full((NCORE, E1S), pl.ZIT, np.int64)
    idx1[ec, pos1] = itrow[it1]
    pl.a2u = (ec, elane, cum1[eblk] + et)        # alpha coords for adj[:, :E1]

    padc1 = np.zeros((NCORE, NB1, P), np.int64)
    np.add.at(padc1, (ec, eblk, elane), 1)
    padc1 = T1[None, :, None] - padc1            # pads per (core, block, lane)

    # ---- P2 item-dst half (dst = item, col = user) --------------------------
    ec2 = core_i[it1]
    elane2 = slot_i[it1] % P
    eblk2 = blk_i[it1]
    isA = colA
    # cumcount within (item, section)
    key2 = it1 * 2 + (~isA).astype(np.int64)
    et2 = _cumcount(key2)
    idx2A = np.full((NCORE, E2AS), pl.ZUL, np.int64)
    idx2B = np.full((NCORE, E2BS), pl.ZUH, np.int64)
    mA = isA
    mB = ~isA
    posA = off2A[eblk2[mA]] + et2[mA] * P + elane2[mA]
    idx2A[ec2[mA], posA] = ug[u1[mA]]
    posB = off2B[eblk2[mB]] + et2[mB] * P + elane2[mB]
    idx2B[ec2[mB], posB] = ug[u1[mB]] - pl.ULO
    acol2 = np.where(isA, cum2[eblk2] + et2,
                     cum2[eblk2] + T2A[eblk2] + et2)
    pl.a2i = (ec2, elane2, acol2)                # alpha coords for adj[:, E1:]

    padc2 = np.zeros((NCORE, NBI, P), np.int64)
    np.add.at(padc2, (ec2, eblk2, elane2), 1)
    padc2 = (T2A + T2B)[None, :, None] - padc2

    # ---- wrapped int16 index tensors ---------------------------------------
    I1 = np.concatenate(
        [np.stack([_wrap_idx(idx1[c, off1[b]:off1[b + 1]]) for c in range(NCORE)])
         for b in range(NB1)], axis=2)           # [NCORE, 128, sum 8*T1]
    I2 = []
    for b in range(NBI):
        I2.append(np.stack([_wrap_idx(idx2A[c, off2A[b]:off2A[b + 1]]) for c in range(NCORE)]))
        I2.append(np.stack([_wrap_idx(idx2B[c, off2B[b]:off2B[b + 1]]) for c in range(NCORE)]))
    I2 = np.concatenate(I2, axis=2)              # [NCORE, 128, sum 8*(T2A+T2B)]
    pl.NI1, pl.NI2 = I1.shape[2], I2.shape[2]

    # ---- per-core dense inputs ---------------------------------------------
    KP = -(-(DF + 1) // P) * P
    pl.KP = KP
    W_pad = np.zeros((KP, DC), np.float32)
    W_pad[:DF] = np.asarray(mlp_w, np.float32)
    W_pad[DF] = np.asarray(mlp_b, np.float32)

    F = np.zeros((NCORE * IPC, KP), np.float32)
    fr = np.asarray(features, np.float32)
    F[itrow, :DF] = fr
    F[itrow, DF] = 1.0

    prefs = np.zeros((NCORE, UPC, DC), np.float32)
    prefs[core_u, slot_u] = np.asarray(preference, np.float32)

    padc = np.concatenate([padc1, padc2], axis=1)          # [NCORE, NB1+NBI, 128]
    pl.padc = np.ascontiguousarray(
        padc.transpose(0, 2, 1)).astype(np.float32)        # [NCORE, 128, NB]

    pl.in_maps = []
    for c in range(NCORE):
        pl.in_maps.append({
            "features_t": np.ascontiguousarray(F[c * IPC:(c + 1) * IPC].T),
            "w_pad": W_pad,
            "pref": prefs[c],
            "idx1": np.ascontiguousarray(I1[c]),
            "idx2": np.ascontiguousarray(I2[c]),
            "padc": pl.padc[c],
        })
    return pl


# ----------------------------------------------------------------------------
# device program
# ----------------------------------------------------------------------------

def _l2norm_rows(nc, pools, x_sb, out_sb):
    """out = x / max(||x||, ~0) rowwise for [128, 64] tiles."""
    ss = pools["small"].tile([P, 1], mybir.dt.float32, tag="ss")
    sq = pools["small"].tile([P, 64], mybir.dt.float32, tag="sq")
    nc.scalar.activation(out=sq[:], in_=x_sb[:],
                         func=mybir.ActivationFunctionType.Square)
    nc.vector.tensor_reduce(out=ss[:], in_=sq[:],
                            axis=mybir.AxisListType.X, op=mybir.AluOpType.add)
    sn = pools["small"].tile([P, 1], mybir.dt.float32, tag="sn")
    nc.scalar.activation(out=sn[:], in_=ss[:],
                         func=mybir.ActivationFunctionType.Sqrt,
                         bias=pools["eps24"][:, :1])
    rn = pools["small"].tile([P, 1], mybir.dt.float32, tag="rn")
    nc.vector.reciprocal(out=rn[:], in_=sn[:])
    nc.vector.tensor_scalar_mul(out=out_sb[:], in0=x_sb[:], scalar1=rn[:, :1])


def _build(pl):
    NB1, NBI = pl.NB1, pl.NBI
    UPC, IPC, KP, DC = pl.UPC, pl.IPC, pl.KP, pl.DC
    T1, T2A, T2B = pl.T1, pl.T2A, pl.T2B
    NROW_IT = NCORE * IPC + 1
    NROW_UL = pl.ULO + 1
    NROW_UH = pl.UHI + 1

    nc = bacc.Bacc("TRN2", target_bir_lowering=False, debug=False,
                   num_devices=NCORE, num_swdge_queues=4)
    features_t = nc.dram_tensor("features_t", [KP, IPC], mybir.dt.float32, kind="ExternalInput")
    w_pad = nc.dram_tensor("w_pad", [KP, DC], mybir.dt.float32, kind="ExternalInput")
    pref_in = nc.dram_tensor("pref", [UPC, DC], mybir.dt.float32, kind="ExternalInput")
    idx1_in = nc.dram_tensor("idx1", [P, pl.NI1], mybir.dt.int16, kind="ExternalInput")
    idx2_in = nc.dram_tensor("idx2", [P, pl.NI2], mybir.dt.int16, kind="ExternalInput")
    padc_in = nc.dram_tensor("padc", [P, NB1 + NBI], mybir.dt.float32, kind="ExternalInput")
    out_nodes = nc.dram_tensor("out_nodes", [UPC + IPC, DC], mybir.dt.float32, kind="ExternalOutput")
    out_alpha = nc.dram_tensor("out_alpha", [P, pl.AW], mybir.dt.float32, kind="ExternalOutput")

    t_items = nc.dram_tensor("t_items", [NROW_IT, DC], mybir.dt.float32)
    t_ulo = nc.dram_tensor("t_ulo", [NROW_UL, DC], mybir.dt.float32)
    t_uhi = nc.dram_tensor("t_uhi", [NROW_UH, DC], mybir.dt.float32)
    ubounce = nc.dram_tensor("ubounce", [NCORE * UPC, DC], mybir.dt.float32)
    mlp_out = nc.dram_tensor("mlp_out", [IPC, DC], mybir.dt.float32)
    pref_a = nc.dram_tensor("pref_a", [UPC, DC], mybir.dt.float32)
    pref_b = nc.dram_tensor("pref_b", [UPC, DC], mybir.dt.float32)

    with tile.TileContext(nc) as tc:
        pools = {}
        from contextlib import ExitStack
        ctx = ExitStack()
        pools["const"] = ctx.enter_context(tc.tile_pool(name="const", bufs=1))
        pools["io"] = ctx.enter_context(tc.tile_pool(name="io", bufs=3))
        pools["g"] = ctx.enter_context(tc.tile_pool(name="g", bufs=2))
        pools["work"] = ctx.enter_context(tc.tile_pool(name="work", bufs=2))
        pools["small"] = ctx.enter_context(tc.tile_pool(name="small", bufs=3))
        pools["psum"] = ctx.enter_context(tc.tile_pool(name="psum", bufs=1, space="PSUM"))

        # constants
        wt = pools["const"].tile([P, (KP // P) * DC], mybir.dt.float32, tag="wt")
        nc.sync.dma_start(out=wt[:].rearrange("p (k n) -> p k n", n=DC),
                          in_=w_pad[:, :].rearrange("(k p) n -> p k n", p=P))
        padc_t = pools["const"].tile([P, NB1 + NBI], mybir.dt.float32, tag="padc")
        nc.sync.dma_start(out=padc_t[:], in_=padc_in[:, :])
        i1_t = pools["const"].tile([P, pl.NI1], mybir.dt.int16, tag="i1")
        nc.sync.dma_start(out=i1_t[:], in_=idx1_in[:, :])
        i2_t = pools["const"].tile([P, pl.NI2], mybir.dt.int16, tag="i2")
        nc.sync.dma_start(out=i2_t[:], in_=idx2_in[:, :])
        eps24 = pools["const"].tile([P, 1], mybir.dt.float32, tag="eps24")
        nc.vector.memset(eps24[:], 1e-24)
        pools["eps24"] = eps24
        zt = pools["const"].tile([P, DC], mybir.dt.float32, tag="zt")
        nc.vector.memset(zt[:], 0.0)
        nc.sync.dma_start(out=t_items[NROW_IT - 1:NROW_IT, :], in_=zt[:1, :])
        nc.sync.dma_start(out=t_ulo[NROW_UL - 1:NROW_UL, :], in_=zt[:1, :])
        nc.sync.dma_start(out=t_uhi[NROW_UH - 1:NROW_UH, :], in_=zt[:1, :])

        # ---- P0: item MLP + leaky_relu + l2norm -----------------------------
        NK = KP // P if "p0" in DBG else 0
        MB = IPC // P                      # 128-row blocks of items
        GRP = 6
        for mg0 in range(0, MB, GRP):
            js = list(range(mg0, min(mg0 + GRP, MB)))
            ps = {}
            for j in js:
                ps_tile = pools["psum"].tile([P, DC], mybir.dt.float32,
                                             tag=f"ps{j - mg0}", name=f"ps_{j}")
                ps[j] = ps_tile
            for k in range(NK):
                xt_k = pools["io"].tile([P, len(js) * P], mybir.dt.float32, tag="xtk")
                nc.sync.dma_start(
                    out=xt_k[:],
                    in_=features_t[k * P:(k + 1) * P, mg0 * P:(mg0 + len(js)) * P])
                for ji, j in enumerate(js):
                    nc.tensor.matmul(
                        out=ps[j][:],
                        lhsT=xt_k[:, ji * P:(ji + 1) * P],
                        rhs=wt[:, k * DC:(k + 1) * DC],
                        start=(k == 0), stop=(k == NK - 1),
                    )
            for j in js:
                f_sb = pools["work"].tile([P, DC], mybir.dt.float32, tag="f_sb")
                fs_s = pools["work"].tile([P, DC], mybir.dt.float32, tag="fs_s")
                nc.vector.tensor_scalar_mul(out=fs_s[:], in0=ps[j][:], scalar1=NEG_SLOPE)
                nc.vector.tensor_tensor(out=f_sb[:], in0=ps[j][:], in1=fs_s[:],
                                        op=mybir.AluOpType.max)
                fn = pools["work"].tile([P, DC], mybir.dt.float32, tag="fn")
                _l2norm_rows(nc, pools, f_sb, fn)
                nc.sync.dma_start(out=mlp_out[j * P:(j + 1) * P, :], in_=fn[:])

        if "ag1" in DBG:
            nc.gpsimd.collective_compute(
                "AllGather", mybir.AluOpType.bypass,
                replica_groups=[list(range(NCORE))],
                ins=[mlp_out[:, :]], outs=[t_items[0:NCORE * IPC, :]],
            )

        # ---- P0b: normalize initial preferences -----------------------------
        for b in range(NB1 if "p0b" in DBG else 0):
            xb = pools["io"].tile([P, DC], mybir.dt.float32, tag="xb0")
            nc.sync.dma_start(out=xb[:], in_=pref_in[b * P:(b + 1) * P, :])
            pn = pools["work"].tile([P, DC], mybir.dt.float32, tag="pn0")
            _l2norm_rows(nc, pools, xb, pn)
            nc.sync.dma_start(out=pref_a[b * P:(b + 1) * P, :], in_=pn[:])

        # ---- shared edge-block body -----------------------------------------
        def edge_block(gathers, xb_src, padcol, alpha_cols, qn):
            """gathers: list of (table_ap, idx_tile_slice, T). Returns (w, G3, xb)."""
            T = sum(t for _, _, t in gathers)
            G = pools["g"].tile([P, T * DC], mybir.dt.float32, tag="G")
            coff = 0
            for (tab, isl, t), q in zip(gathers, qn):
                nc.gpsimd.dma_gather(
                    G[:, coff * DC:(coff + t) * DC].rearrange("p (t f) -> p t f", f=DC),
                    tab, isl, P * t, P * t, DC,
                    single_packet=False, queue_num=q,
                )
                coff += t
            xb = pools["io"].tile([P, DC], mybir.dt.float32, tag="xb")
            nc.sync.dma_start(out=xb[:], in_=xb_src)
            G3 = G[:].rearrange("p (t f) -> p t f", f=DC)
            prod = pools["g"].tile([P, T * DC], mybir.dt.float32, tag="prod")
            nc.vector.tensor_tensor(
                out=prod[:].rearrange("p (t f) -> p t f", f=DC),
                in0=G3, in1=xb[:, None, :].to_broadcast([P, T, DC]),
                op=mybir.AluOpType.mult,
            )
            al = pools["work"].tile([P, T], mybir.dt.float32, tag="al")
            nc.vector.tensor_reduce(
                out=al[:], in_=prod[:].rearrange("p (t f) -> p t f", f=DC),
                axis=mybir.AxisListType.X, op=mybir.AluOpType.add,
            )
            e = pools["work"].tile([P, T], mybir.dt.float32, tag="e")
            nc.scalar.activation(out=e[:], in_=al[:],
                                 func=mybir.ActivationFunctionType.Exp)
            sraw = pools["small"].tile([P, 1], mybir.dt.float32, tag="sraw")
            nc.vector.tensor_reduce(out=sraw[:], in_=e[:],
                                    axis=mybir.AxisListType.X, op=mybir.AluOpType.add)
            sfix = pools["small"].tile([P, 1], mybir.dt.float32, tag="sfix")
            nc.vector.tensor_tensor(out=sfix[:], in0=sraw[:], in1=padcol,
                                    op=mybir.AluOpType.subtract)
            sfx2 = pools["small"].tile([P, 1], mybir.dt.float32, tag="sfx2")
            nc.vector.tensor_scalar_max(out=sfx2[:], in0=sfix[:], scalar1=EPS_SOFT)
            rs = pools["small"].tile([P, 1], mybir.dt.float32, tag="rs")
            nc.vector.reciprocal(out=rs[:], in_=sfx2[:])
            w = pools["work"].tile([P, T], mybir.dt.float32, tag="w")
            nc.vector.tensor_scalar_mul(out=w[:], in0=e[:], scalar1=rs[:, :1])
            wg = pools["g"].tile([P, T * DC], mybir.dt.float32, tag="wg")
            nc.vector.tensor_tensor(
                out=wg[:].rearrange("p (t f) -> p t f", f=DC),
                in0=G3, in1=w[:, :, None].to_broadcast([P, T, DC]),
                op=mybir.AluOpType.mult,
            )
            outb = pools["work"].tile([P, DC], mybir.dt.float32, tag="outb")
            nc.vector.tensor_reduce(
                out=outb[:], in_=wg[:].rearrange("p (t f) -> p f t", f=DC),
                axis=mybir.AxisListType.X, op=mybir.AluOpType.add,
            )
            if alpha_cols is not None:
                nc.sync.dma_start(
                    out=out_alpha[:, alpha_cols[0]:alpha_cols[0] + T], in_=w[:])
            return outb, xb

        # ---- P1: three routing steps over adj_user --------------------------
        bufs = [pref_a, pref_b]
        for step in range(3 if "p1" in DBG else 0):
            src = bufs[step % 2]
            dst = bufs[(step + 1) % 2]
            for b in range(NB1):
                t = int(T1[b])
                i1o = 8 * int(pl.cum1[b])
                outb, xb = edge_block(
                    [(t_items[:, :], i1_t[:, i1o:i1o + 8 * t], t)],
                    src[b * P:(b + 1) * P, :],
                    padc_t[:, b:b + 1], None, [b % 4],
                )
                p2 = pools["work"].tile([P, DC], mybir.dt.float32, tag="p2")
                nc.vector.tensor_add(out=p2[:], in0=xb[:], in1=outb[:])
                pn = pools["work"].tile([P, DC], mybir.dt.float32, tag="pn")
                _l2norm_rows(nc, pools, p2, pn)
                nc.sync.dma_start(out=dst[b * P:(b + 1) * P, :], in_=pn[:])

        pref_fin = bufs[3 % 2]  # after 3 steps: a->b, b->a, a->b => pref_b
        if "ag2" in DBG:
            nc.gpsimd.collective_compute(
                "AllGather", mybir.AluOpType.bypass,
                replica_groups=[list(range(NCORE))],
                ins=[pref_fin[:, :]], outs=[ubounce[:, :]],
            )
            nc.sync.dma_start(out=t_ulo[0:pl.ULO, :], in_=ubounce[0:pl.ULO, :])
            nc.sync.dma_start(out=t_uhi[0:pl.UHI, :], in_=ubounce[pl.ULO:, :])

        # ---- P2: final pass over adj ---------------------------------------
        def final_block(gathers, xb_src, padcol, acol, outrow, qn):
            outb, xb = edge_block(gathers, xb_src, padcol, (acol,), qn)
            xh = pools["work"].tile([P, DC], mybir.dt.float32, tag="xh")
            xh_s = pools["work"].tile([P, DC], mybir.dt.float32, tag="xh_s")
            nc.vector.tensor_scalar_mul(out=xh_s[:], in0=outb[:], scalar1=NEG_SLOPE)
            nc.vector.tensor_tensor(out=xh[:], in0=outb[:], in1=xh_s[:],
                                    op=mybir.AluOpType.max)
            on = pools["work"].tile([P, DC], mybir.dt.float32, tag="on")
            nc.vector.tensor_add(out=on[:], in0=xb[:], in1=xh[:])
            nc.sync.dma_start(out=out_nodes[outrow:outrow + P, :], in_=on[:])

        for b in range(NB1 if "p2" in DBG else 0):
            t = int(T1[b])
            i1o = 8 * int(pl.cum1[b])
            final_block(
                [(t_items[:, :], i1_t[:, i1o:i1o + 8 * t], t)],
                pref_fin[b * P:(b + 1) * P, :],
                padc_t[:, b:b + 1], int(pl.cum1[b]), b * P, [b % 4],
            )
        for bi in range(NBI if "p2" in DBG else 0):
            ta, tb = int(T2A[bi]), int(T2B[bi])
            i2o = 8 * (int(pl.cum2[bi]) - int(pl.cum1[-1]))
            final_block(
                [(t_ulo[:, :], i2_t[:, i2o:i2o + 8 * ta], ta),
                 (t_uhi[:, :], i2_t[:, i2o + 8 * ta:i2o + 8 * (ta + tb)], tb)],
                mlp_out[bi * P:(bi + 1) * P, :],
                padc_t[:, NB1 + bi:NB1 + bi + 1],
                int(pl.cum2[bi]), (NB1 + bi) * P, [bi % 4, (bi + 1) % 4],
            )
        ctx.close()
    nc.compile()
    return nc


# ----------------------------------------------------------------------------
# entry point
# ----------------------------------------------------------------------------

class _Runner:
    def __init__(self, nc, n_cores):
        import jax
        from jax.sharding import Mesh, PartitionSpec
        from jax.experimental.shard_map import shard_map
        from concourse.bass2jax import (_bass_exec_p, partition_id_tensor,
                                        install_neuronx_cc_hook)
        install_neuronx_cc_hook()
        self.jax = jax
        self.n_cores = n_cores
        in_names, out_names, out_avals, zero_outs = [], [], [], []
        partition_name = nc.partition_id_tensor.name if nc.partition_id_tensor else None
        for alloc in nc.m.functions[0].allocations:
            if not isinstance(alloc, mybir.MemoryLocationSet):
                continue
            name = alloc.memorylocations[0].name
            if alloc.kind == "ExternalInput":
                if name != partition_name:
                    in_names.append(name)
            elif alloc.kind == "ExternalOutput":
                out_avals.append(jax.core.ShapedArray(
                    tuple(alloc.tensor_shape), mybir.dt.np(alloc.dtype)))
                out_names.append(name)
                zero_outs.append(np.zeros(tuple(alloc.tensor_shape),
                                          mybir.dt.np(alloc.dtype)))
        self.in_names, self.out_names = in_names, out_names
        self.out_avals, self.zero_outs = out_avals, zero_outs
        n_params, n_outs = len(in_names), len(out_names)
        all_in_names = list(in_names) + list(out_names)
        if partition_name is not None:
            all_in_names.append(partition_name)

        def _body(*args):
            operands = list(args)
            if partition_name is not None:
                operands.append(partition_id_tensor())
            return tuple(_bass_exec_p.bind(
                *operands, out_avals=tuple(out_avals), in_names=tuple(all_in_names),
                out_names=tuple(out_names), lowering_input_output_aliases=(),
                sim_require_finite=True, sim_require_nnan=True, nc=nc))

        devices = jax.devices()[:n_cores]
        donate = tuple(range(n_params, n_params + n_outs))
        mesh = Mesh(np.asarray(devices), ("core",))
        in_specs = (PartitionSpec("core"),) * (n_params + n_outs)
        out_specs = (PartitionSpec("core"),) * n_outs
        self.fn = jax.jit(
            shard_map(_body, mesh=mesh, in_specs=in_specs,
                      out_specs=out_specs, check_rep=False),
            donate_argnums=donate, keep_unused=True)

    def prep_inputs(self, in_maps):
        args = [np.concatenate([np.asarray(m[n]) for m in in_maps], axis=0)
                for n in self.in_names]
        return [self.jax.device_put(a) for a in args]

    def run(self, dev_args):
        zeros = [np.zeros((self.n_cores * z.shape[0], *z.shape[1:]), z.dtype)
                 for z in self.zero_outs]
        outs = self.fn(*dev_args, *zeros)
        self.jax.block_until_ready(outs)
        return outs

    def results(self, outs):
        res = []
        for c in range(self.n_cores):
            d = {}
            for i, name in enumerate(self.out_names):
                a = np.asarray(outs[i]).reshape(self.n_cores, *self.out_avals[i].shape)
                d[name] = a[c]
            res.append(d)
        return res


def _get_runner(pl):
    nc = _build(pl)
    return _Runner(nc, NCORE)


def _assemble(pl, res, adj, adj_user):
    NU, NI = pl.NU, pl.NI
    out_full = np.empty((NU + NI, pl.DC), np.float32)
    nodes = np.stack([res[c]["out_nodes"] for c in range(NCORE)])  # [8, UPC+IPC, 64]
    out_full[:NU] = nodes[pl.core_u, pl.slot_u]
    out_full[NU:] = nodes[pl.core_i, pl.UPC + pl.slot_i]

    E1 = adj_user.shape[1]
    alpha = np.empty(adj.shape[1], np.float32)
    oa = np.stack([res[c]["out_alpha"] for c in range(NCORE)])     # [8, 128, AW]
    ec, elane, ecol = pl.a2u
    alpha[:E1] = oa[ec, elane, ecol]
    ec2, elane2, ecol2 = pl.a2i
    alpha[E1:] = oa[ec2, elane2, ecol2]
    return out_full, alpha[:, None]


_LAST_PLAN = None
_LAST_RUNNER = None


def kernel(features, mlp_w, mlp_b, preference, adj, adj_user):
    global _LAST_PLAN, _LAST_RUNNER
    features = np.asarray(features)
    mlp_w = np.asarray(mlp_w)
    mlp_b = np.asarray(mlp_b)
    preference = np.asarray(preference)
    adj = np.asarray(adj)
    adj_user = np.asarray(adj_user)

    pl = _preprocess(features, mlp_w, mlp_b, preference, adj, adj_user)
    runner = _get_runner(pl)
    _LAST_PLAN, _LAST_RUNNER = pl, runner
    dev = runner.prep_inputs(pl.in_maps)
    outs = runner.run(dev)
    res = runner.results(outs)
    return _assemble(pl, res, adj, adj_user)
